# revision 1
# baseline (speedup 1.0000x reference)
"""DispNetC correlation volume on 8 NeuronCores (Trainium2, Bass/Tile).

out[b, d, h, w] = mean_c(L[b,c,h,w] * R[b,c,h,w-d]), d in [0,40), 0 where w<d.

Sharding: data-parallel over batch (B=8 -> 1 sample per core). Per core:

1. Stream L,R in h-blocks to SBUF as [c_lo(128 part), (c_hi, h, w)].
2. Per h: 2 accumulating fp32 matmuls -> PSUM Gram G[w, w'] = sum_c L[c,w]R[c,w'].
   The needed band is out[d, w] = G[w, w-d]/256 - 40 diagonals.
3. Diagonals are partition-coupled in [w, w'] layout, so re-layout Grams to
   h-on-partition form: DVE-copy (x 1/256) Grams into G_all[w, h*128+w'],
   DMA band strips (4 per Gram, 32 w-rows x 71 w'-window) to DRAM scratch,
   DMA back as X[p=2h+q, s2*2272 + 71*i + j] (q = w-half, s2 = w-quarter).
   Clipped strip regions are pre-zeroed -> w<d positions read zeros.
4. In X, diagonal d for ALL h is an uncoupled strided AP (free step 72):
   one DVE copy per d -> O[p, 64*d + 32*s2 + i].
5. Two DMAs (one per q) write O to out[d, h, w] with 512B-contiguous runs.
"""

import numpy as np

C, H, W, D = 256, 64, 128, 40
NS = 71                  # strip window width (39 + 32)
SFREE = 32 * NS          # 2272
XF = 2 * SFREE           # X free size
F3 = D * 64              # O free size
HG = 32                  # h per group
NGRP = 2
HB = 8                   # h per input block
J0 = [39, 7, 0, 0]       # clipped leading j per strip
N_CORES = 8

_cache = {}


def _build():
    import concourse.bacc as bacc
    import concourse.mybir as mybir
    from concourse.tile import TileContext

    f32 = mybir.dt.float32
    nc = bacc.Bacc("TRN2", target_bir_lowering=False, debug=False,
                   num_devices=N_CORES)
    l_in = nc.dram_tensor("l", [C, H, W], f32, kind="ExternalInput")
    r_in = nc.dram_tensor("r", [C, H, W], f32, kind="ExternalInput")
    out = nc.dram_tensor("out", [D, H, W], f32, kind="ExternalOutput")

    with TileContext(nc) as tc:
        with (
            tc.tile_pool(name="inp", bufs=3) as inp,
            tc.tile_pool(name="gall", bufs=2) as gallp,
            tc.tile_pool(name="fix", bufs=1) as fix,
            tc.tile_pool(name="ps", bufs=6, space="PSUM") as psp,
            tc.tile_pool(name="dram", bufs=1, space="DRAM") as dp,
        ):
            x_t = fix.tile([128, XF], f32, tag="x")
            o_t = fix.tile([128, F3], f32, tag="o")
            scratch = dp.tile([NGRP, 4, 32, HG, NS], f32)

            lv = l_in.ap().rearrange("(ch p) h w -> p ch h w", ch=2)
            rv = r_in.ap().rearrange("(ch p) h w -> p ch h w", ch=2)

            for g in range(NGRP):
                g_all = gallp.tile([128, HG * W], f32, tag="gall")
                ga3 = g_all[:, :].rearrange("w (h x) -> w h x", x=W)
                for blk in range(HG // HB):
                    h0 = g * HG + blk * HB
                    lt = inp.tile([128, 2 * HB * W], f32, tag="lt")
                    rt = inp.tile([128, 2 * HB * W], f32, tag="rt")
                    lt4 = lt[:, :].rearrange("p (ch h w) -> p ch h w", ch=2, h=HB)
                    rt4 = rt[:, :].rearrange("p (ch h w) -> p ch h w", ch=2, h=HB)
                    nc.sync.dma_start(lt4, lv[:, :, h0 : h0 + HB, :])
                    nc.sync.dma_start(rt4, rv[:, :, h0 : h0 + HB, :])
                    for hb in range(HB):
                        h_loc = blk * HB + hb
                        gm = psp.tile([128, W], f32, tag="gram")
                        for ch in range(2):
                            nc.tensor.matmul(
                                gm[:, :], lt4[:, ch, hb, :], rt4[:, ch, hb, :],
                                start=(ch == 0), stop=(ch == 1),
                            )
                        nc.vector.tensor_scalar_mul(
                            ga3[:, h_loc, :], gm[:, :], 1.0 / C,
                        )

                # band strips -> DRAM scratch S[g, s, i, h, j]
                for s in range(4):
                    j0 = J0[s]
                    nj = NS - j0
                    wp0 = 32 * s - 39 + j0
                    nc.sync.dma_start(
                        scratch[g, s, :, :, j0:],
                        ga3[32 * s : 32 * s + 32, :, wp0 : wp0 + nj],
                    )

                # X partitions [64g, 64g+64): p = 64g + 2h_loc + q
                xg = x_t[64 * g : 64 * g + 64, :]
                x4 = xg.rearrange("p (a i j) -> p a i j", a=2, j=NS)
                nc.vector.memset(x4[:, 0, :, 0:39], 0.0)
                nc.vector.memset(x4[:, 1, :, 0:7], 0.0)
                xq = xg.rearrange("(h two) (a i j) -> h two a i j",
                                  two=2, a=2, j=NS)
                for s in range(4):
                    q, s2 = s >> 1, s & 1
                    j0 = J0[s]
                    nc.sync.dma_start(
                        xq[:, q, s2, :, j0:],
                        scratch[g, s, :, :, j0:].transpose([1, 0, 2]),
                    )

                # per-diagonal extraction
                xs = xg.rearrange("p (a f) -> p a f", a=2)
                ovw = o_t[64 * g : 64 * g + 64, :].rearrange(
                    "p (d a i) -> p d a i", d=D, a=2)
                for d in range(D):
                    lo = 39 - d
                    nc.vector.tensor_copy(
                        ovw[:, d, :, :], xs[:, :, lo : lo + 72 * 31 + 1 : 72])

            # out DMAs: one per q (single partition-step dim each)
            srcq = o_t[:, :].rearrange("(h two) (d w) -> h two d w", two=2, d=D)
            dstq = out.ap().rearrange("d h (two w) -> h two d w", two=2)
            for q in range(2):
                nc.sync.dma_start(dstq[:, q], srcq[:, q])

    nc.compile()
    return nc


def _get_program():
    if "nc" not in _cache:
        _cache["nc"] = _build()
    return _cache["nc"]


def kernel(conv3a_l: np.ndarray, conv3a_r: np.ndarray) -> np.ndarray:
    from concourse import bass_utils

    nc = _get_program()
    conv3a_l = np.ascontiguousarray(conv3a_l, dtype=np.float32)
    conv3a_r = np.ascontiguousarray(conv3a_r, dtype=np.float32)
    in_maps = [
        {"l": conv3a_l[b], "r": conv3a_r[b]} for b in range(N_CORES)
    ]
    res = bass_utils.run_bass_kernel_spmd(nc, in_maps,
                                          core_ids=list(range(N_CORES)))
    return np.stack([res.results[b]["out"] for b in range(N_CORES)], axis=0)
